# revision 2
# baseline (speedup 1.0000x reference)
"""Trainium2 Bass kernel for nn_DGM_d (gumbel top-k over centered pairwise
squared distances).

Full inputs in, full outputs out. Internally shards the node dimension n
across 8 NeuronCores (512 rows per core per batch); each core computes its
[b, 512, 4096] perturbed-logit rows and does an exact fp32 top-16 per row
(chunked max8 + merge + full-width max_index), then the host concatenates
row shards and assembles the sparse edge list.
"""

import sys

if '/opt/trn_rl_repo' not in sys.path:
    sys.path.insert(0, '/opt/trn_rl_repo')

import numpy as np

B, N, D_FEAT, K = 4, 4096, 128, 16
NCORES = 8
RPC = N // NCORES          # rows per core per batch (512)
RT = RPC // 128            # row-tiles per batch per core (4)

_CACHE = {}


def _build(c_temp: float):
    import concourse.bass as bass  # noqa: F401
    import concourse.mybir as mybir
    from concourse import bacc
    from concourse.tile import TileContext

    f32 = mybir.dt.float32
    u32 = mybir.dt.uint32
    ALU = mybir.AluOpType
    ACT = mybir.ActivationFunctionType

    nc = bacc.Bacc()
    xT_d = nc.dram_tensor("xT", [B, 128, N], f32, kind="ExternalInput")
    q_d = nc.dram_tensor("qs", [B, RPC, N], f32, kind="ExternalInput")
    eye_d = nc.dram_tensor("eye", [128, 128], f32, kind="ExternalInput")
    eyeneg_d = nc.dram_tensor("eyeneg", [128, 128], f32, kind="ExternalInput")
    lp_d = nc.dram_tensor("lp", [B, RPC, K], f32, kind="ExternalOutput")
    idx_d = nc.dram_tensor("idx", [B, RPC, K], u32, kind="ExternalOutput")

    with TileContext(nc) as tc:
        with (
            tc.tile_pool(name="sb", bufs=1) as pc,      # consts / per-batch
            tc.tile_pool(name="bt", bufs=2) as pbt,     # xcT/xcT2 (double-buffered)
            tc.tile_pool(name="big", bufs=2) as pb,     # per-tile big buffers
            tc.tile_pool(name="sm", bufs=2) as psm,     # per-tile small buffers
            tc.tile_pool(name="ps", bufs=2, space="PSUM") as pp,
        ):
            eye = pc.tile([128, 128], f32)
            eyeneg = pc.tile([128, 128], f32)
            ones = pc.tile([128, 1], f32)
            ones_row = pc.tile([1, 128], f32)
            c1e8 = pc.tile([128, 1], f32)
            nc.sync.dma_start(eye[:], eye_d[:])
            nc.sync.dma_start(eyeneg[:], eyeneg_d[:])
            nc.vector.memset(ones[:], 1.0)
            nc.vector.memset(ones_row[:], 1.0)
            nc.vector.memset(c1e8[:], 1e-8)

            for b in range(B):
                # ---------- per-batch prologue ----------
                xT = pc.tile([128, N], f32, tag="xa")
                nc.sync.dma_start(xT[:], xT_d[b])
                mu = pc.tile([128, 1], f32, tag="mu")
                nc.vector.reduce_sum(mu[:], xT[:], axis=mybir.AxisListType.X)
                nc.vector.tensor_scalar_mul(mu[:], mu[:], 1.0 / N)
                xcT = pbt.tile([128, N], f32, tag="xc")
                nc.vector.tensor_scalar(xcT[:], xT[:], mu[:], None, op0=ALU.subtract)
                xcT2 = pbt.tile([128, N], f32, tag="x2")
                nc.vector.tensor_scalar_mul(xcT2[:], xcT[:], 2.0 * c_temp)
                sqel = pc.tile([128, N], f32, tag="xa")  # reuses xT slot
                nc.scalar.activation(sqel[:], xcT[:], ACT.Square)
                negsq = pc.tile([1, N], f32, tag="nsq")
                for k in range(8):
                    sl = slice(k * 512, (k + 1) * 512)
                    psA = pp.tile([128, 512], f32, tag="mm")
                    nc.tensor.matmul(psA[0:1, :], ones[:], sqel[:, sl],
                                     start=True, stop=True)
                    nc.scalar.activation(negsq[:, sl], psA[0:1, :], ACT.Copy,
                                         scale=-c_temp)

                for t in range(RT):
                    # global column base of this row block's diagonal.
                    # rows handled here are [core*RPC + t*128, ... + 128); the
                    # core offset enters via a per-core column rotation of the
                    # data (see host side) so the built program is identical on
                    # all cores: the diagonal always sits at column t*128 of
                    # the rotated layout.
                    gb = t * 128

                    # ---------- sq_i for this row block ----------
                    psA = pp.tile([128, 512], f32, tag="mm")
                    nc.tensor.transpose(psA[:, 0:128], xcT[:, gb:gb + 128], eye[:])
                    sqr = psm.tile([128, 128], f32, tag="sqr")
                    nc.scalar.activation(sqr[:], psA[:, 0:128], ACT.Square)
                    sq_ic = psm.tile([128, 1], f32, tag="sqi")
                    nc.vector.reduce_sum(sq_ic[:], sqr[:], axis=mybir.AxisListType.X)
                    nc.vector.tensor_scalar_mul(sq_ic[:], sq_ic[:], c_temp)

                    # ---------- gumbel ----------
                    qt = pb.tile([128, N], f32, tag="qt")
                    nc.sync.dma_start(qt[:], q_d[b, t * 128:(t + 1) * 128])
                    tt = pb.tile([128, N], f32, tag="ttlq")
                    nc.scalar.activation(tt[:], qt[:], ACT.Ln, bias=c1e8[:])
                    g = pb.tile([128, N], f32, tag="g")
                    nc.scalar.activation(g[:], tt[:], ACT.Ln, scale=-1.0)

                    # ---------- matmul + combine, two 2048 halves ----------
                    lq = pb.tile([128, N], f32, tag="ttlq")
                    for h in range(2):
                        ps = pp.tile([128, 2048], f32, tag="mm")
                        h0 = h * 2048
                        for k in range(4):
                            lsl = slice(k * 512, (k + 1) * 512)
                            gsl = slice(h0 + k * 512, h0 + (k + 1) * 512)
                            has_diag = (h0 + k * 512) <= gb < (h0 + (k + 1) * 512)
                            nc.tensor.matmul(ps[:, lsl], xcT2[:, gb:gb + 128],
                                             xcT[:, gsl], start=True, stop=False)
                            nc.tensor.matmul(ps[:, lsl], ones_row[:],
                                             negsq[:, gsl], start=False,
                                             stop=not has_diag)
                            if has_diag:
                                off = gb - h0
                                nc.tensor.matmul(ps[:, off:off + 128], eye[:],
                                                 eyeneg[:], start=False, stop=True)
                        nc.vector.tensor_tensor(lq[:, h0:h0 + 2048], ps[:],
                                                g[:, h0:h0 + 2048], ALU.subtract)

                    # ---------- top-16 ----------
                    cand = psm.tile([128, 256], f32, tag="cand")
                    for ch in range(32):
                        nc.vector.max(cand[:, ch * 8:(ch + 1) * 8],
                                      lq[:, ch * 128:(ch + 1) * 128])
                    m1 = psm.tile([128, 8], f32, tag="m1")
                    nc.vector.max(m1[:], cand[:])
                    cand2 = psm.tile([128, 256], f32, tag="cand2")
                    nc.vector.match_replace(cand2[:], m1[:], cand[:], -1e30)
                    m2 = psm.tile([128, 8], f32, tag="m2")
                    nc.vector.max(m2[:], cand2[:])

                    idx = psm.tile([128, K], u32, tag="idx")
                    nc.vector.max_index(idx[:, 0:8], m1[:], lq[:])
                    nc.vector.max_index(idx[:, 8:16], m2[:], lq[:])
                    nc.sync.dma_start(idx_d[b, t * 128:(t + 1) * 128], idx[:])

                    lp = psm.tile([128, K], f32, tag="lp")
                    nc.vector.tensor_scalar(lp[:, 0:8], m1[:], sq_ic[:], None,
                                            op0=ALU.subtract)
                    nc.vector.tensor_scalar(lp[:, 8:16], m2[:], sq_ic[:], None,
                                            op0=ALU.subtract)
                    nc.sync.dma_start(lp_d[b, t * 128:(t + 1) * 128], lp[:])

    nc.finalize()
    return nc


def _get_executor(c_temp: float):
    key = round(float(c_temp), 10)
    if key in _CACHE:
        return _CACHE[key]

    import jax
    import concourse.mybir as mybir
    from concourse.bass2jax import (install_neuronx_cc_hook, _bass_exec_p,
                                    partition_id_tensor)
    from jax.sharding import Mesh, PartitionSpec
    from jax.experimental.shard_map import shard_map

    nc = _build(c_temp)
    install_neuronx_cc_hook()

    partition_name = (nc.partition_id_tensor.name
                      if nc.partition_id_tensor else None)
    in_names, out_names, out_avals, zero_shapes = [], [], [], []
    for alloc in nc.m.functions[0].allocations:
        if not isinstance(alloc, mybir.MemoryLocationSet):
            continue
        name = alloc.memorylocations[0].name
        if alloc.kind == "ExternalInput":
            if name != partition_name:
                in_names.append(name)
        elif alloc.kind == "ExternalOutput":
            out_names.append(name)
            shape = tuple(alloc.tensor_shape)
            dtype = mybir.dt.np(alloc.dtype)
            out_avals.append(jax.core.ShapedArray(shape, dtype))
            zero_shapes.append((shape, dtype))
    n_params = len(in_names)
    all_in_names = in_names + out_names
    if partition_name is not None:
        all_in_names = all_in_names + [partition_name]
    donate = tuple(range(n_params, n_params + len(out_names)))

    def _body(*args):
        operands = list(args)
        if partition_name is not None:
            operands.append(partition_id_tensor())
        return tuple(_bass_exec_p.bind(
            *operands,
            out_avals=tuple(out_avals),
            in_names=tuple(all_in_names),
            out_names=tuple(out_names),
            lowering_input_output_aliases=(),
            sim_require_finite=True,
            sim_require_nnan=True,
            nc=nc,
        ))

    devices = jax.devices()[:NCORES]
    mesh = Mesh(np.asarray(devices), ("core",))
    in_specs = (PartitionSpec("core"),) * (n_params + len(out_names))
    out_specs = (PartitionSpec("core"),) * len(out_names)
    sharded = jax.jit(
        shard_map(_body, mesh=mesh, in_specs=in_specs, out_specs=out_specs,
                  check_rep=False),
        donate_argnums=donate, keep_unused=True)

    exe = {
        "sharded": sharded,
        "in_names": in_names,
        "out_names": out_names,
        "zero_shapes": zero_shapes,
        "out_avals": out_avals,
    }
    _CACHE[key] = exe
    return exe


def _run_device(c_temp, per_core_inputs):
    exe = _get_executor(c_temp)
    concat_in = [
        np.concatenate([per_core_inputs[c][name] for c in range(NCORES)], axis=0)
        for name in exe["in_names"]
    ]
    concat_zeros = [
        np.zeros((NCORES * s[0],) + tuple(s[1:]), d)
        for (s, d) in exe["zero_shapes"]
    ]
    outs = exe["sharded"](*concat_in, *concat_zeros)
    result = []
    for i, name in enumerate(exe["out_names"]):
        a = np.asarray(outs[i])
        shp = exe["out_avals"][i].shape
        result.append((name, a.reshape(NCORES, *shp)))
    return dict(result)


def kernel(x, A, temperature, q):
    x = np.asarray(x, dtype=np.float32)
    q = np.asarray(q, dtype=np.float32)
    tau = np.float32(np.clip(np.asarray(temperature, dtype=np.float32), -4.0, 4.0))
    c_temp = float(np.exp(tau, dtype=np.float32))

    xT = np.ascontiguousarray(np.transpose(x, (0, 2, 1)))          # [B,128,N]
    eye = np.eye(128, dtype=np.float32)
    eyeneg = (-1e20 * np.eye(128)).astype(np.float32)

    per_core = []
    for c in range(NCORES):
        rows = slice(c * RPC, (c + 1) * RPC)
        # rotate the column axis so this core's diagonal block lands at
        # columns [t*128, t*128+128) — keeps the compiled program identical
        # across cores.  column j of the rotated layout = global column
        # (j + c*RPC) % N.
        xT_rot = np.roll(xT, -c * RPC, axis=2)
        q_rot = np.roll(q[:, rows, :], -c * RPC, axis=2)
        per_core.append({
            "xT": np.ascontiguousarray(xT_rot),
            "qs": np.ascontiguousarray(q_rot),
            "eye": eye,
            "eyeneg": eyeneg,
        })

    outs = _run_device(c_temp, per_core)

    lp = outs["lp"]            # [8, B, RPC, K]
    idx = outs["idx"].astype(np.int64)

    logprobs = np.zeros((B, N, K), dtype=np.float32)
    indices = np.zeros((B, N, K), dtype=np.int32)
    for c in range(NCORES):
        rows = slice(c * RPC, (c + 1) * RPC)
        logprobs[:, rows, :] = lp[c]
        # un-rotate the column indices back to global columns
        indices[:, rows, :] = ((idx[c] + c * RPC) % N).astype(np.int32)

    rows_arr = np.broadcast_to(np.arange(N, dtype=np.int32)[None, :, None],
                               (B, N, K))
    edges = np.stack((indices.reshape(B, -1), rows_arr.reshape(B, -1)), axis=-2)
    offset = (np.arange(B, dtype=np.int32) * N)[:, None, None]
    edges_sparse = np.transpose(edges + offset, (1, 0, 2)).reshape(2, -1)
    return x, edges_sparse.astype(np.int32), logprobs


# revision 3
# speedup vs baseline: 35.6334x; 35.6334x over previous
"""Trainium2 Bass kernel for nn_DGM_d (gumbel top-k over centered pairwise
squared distances).

Full inputs in, full outputs out. Internally shards the node dimension n
across 8 NeuronCores (512 rows per core per batch); each core computes its
[b, 512, 4096] perturbed-logit rows and does an exact fp32 top-16 per row
(chunked max8 + merge + full-width max_index), then the host concatenates
row shards and assembles the sparse edge list.
"""

import sys

if '/opt/trn_rl_repo' not in sys.path:
    sys.path.insert(0, '/opt/trn_rl_repo')

import numpy as np

B, N, D_FEAT, K = 4, 4096, 128, 16
NCORES = 8
RPC = N // NCORES          # rows per core per batch (512)
RT = RPC // 128            # row-tiles per batch per core (4)

_CACHE = {}


def _build(c_temp: float):
    import concourse.bass as bass  # noqa: F401
    import concourse.mybir as mybir
    from concourse import bacc
    from concourse.tile import TileContext

    f32 = mybir.dt.float32
    u32 = mybir.dt.uint32
    ALU = mybir.AluOpType
    ACT = mybir.ActivationFunctionType

    nc = bacc.Bacc()
    xT_d = nc.dram_tensor("xT", [B, 128, N], f32, kind="ExternalInput")
    q_d = nc.dram_tensor("qs", [B, RPC, N], f32, kind="ExternalInput")
    eye_d = nc.dram_tensor("eye", [128, 128], f32, kind="ExternalInput")
    eyeneg_d = nc.dram_tensor("eyeneg", [128, 128], f32, kind="ExternalInput")
    lp_d = nc.dram_tensor("lp", [B, RPC, K], f32, kind="ExternalOutput")
    idx_d = nc.dram_tensor("idx", [B, RPC, K], u32, kind="ExternalOutput")

    with TileContext(nc) as tc:
        with (
            tc.tile_pool(name="sb", bufs=1) as pc,      # consts / per-batch
            tc.tile_pool(name="bt", bufs=2) as pbt,     # xcT/xcT2 (double-buffered)
            tc.tile_pool(name="big", bufs=2) as pb,     # per-tile big buffers
            tc.tile_pool(name="sm", bufs=2) as psm,     # per-tile small buffers
            tc.tile_pool(name="ps", bufs=2, space="PSUM") as pp,
        ):
            eye = pc.tile([128, 128], f32)
            eyeneg = pc.tile([128, 128], f32)
            ones = pc.tile([128, 1], f32)
            ones_row = pc.tile([1, 128], f32)
            c1e8 = pc.tile([128, 1], f32)
            nc.sync.dma_start(eye[:], eye_d[:])
            nc.sync.dma_start(eyeneg[:], eyeneg_d[:])
            nc.vector.memset(ones[:], 1.0)
            nc.vector.memset(ones_row[:], 1.0)
            nc.vector.memset(c1e8[:], 1e-8)

            for b in range(B):
                # ---------- per-batch prologue ----------
                xT = pc.tile([128, N], f32, tag="xa")
                nc.sync.dma_start(xT[:], xT_d[b])
                mu = pc.tile([128, 1], f32, tag="mu")
                nc.vector.reduce_sum(mu[:], xT[:], axis=mybir.AxisListType.X)
                nc.vector.tensor_scalar_mul(mu[:], mu[:], 1.0 / N)
                xcT = pbt.tile([128, N], f32, tag="xc")
                nc.vector.tensor_scalar(xcT[:], xT[:], mu[:], None, op0=ALU.subtract)
                xcT2 = pbt.tile([128, N], f32, tag="x2")
                nc.vector.tensor_scalar_mul(xcT2[:], xcT[:], 2.0 * c_temp)
                sqel = pc.tile([128, N], f32, tag="xa")  # reuses xT slot
                nc.scalar.activation(sqel[:], xcT[:], ACT.Square)
                negsq = pc.tile([1, N], f32, tag="nsq")
                for k in range(8):
                    sl = slice(k * 512, (k + 1) * 512)
                    psA = pp.tile([128, 512], f32, tag="mm")
                    nc.tensor.matmul(psA[0:1, :], ones[:], sqel[:, sl],
                                     start=True, stop=True)
                    nc.scalar.activation(negsq[:, sl], psA[0:1, :], ACT.Copy,
                                         scale=-c_temp)

                for t in range(RT):
                    # global column base of this row block's diagonal.
                    # rows handled here are [core*RPC + t*128, ... + 128); the
                    # core offset enters via a per-core column rotation of the
                    # data (see host side) so the built program is identical on
                    # all cores: the diagonal always sits at column t*128 of
                    # the rotated layout.
                    gb = t * 128

                    # ---------- sq_i for this row block ----------
                    psA = pp.tile([128, 512], f32, tag="mm")
                    nc.tensor.transpose(psA[:, 0:128], xcT[:, gb:gb + 128], eye[:])
                    sqr = psm.tile([128, 128], f32, tag="sqr")
                    nc.scalar.activation(sqr[:], psA[:, 0:128], ACT.Square)
                    sq_ic = psm.tile([128, 1], f32, tag="sqi")
                    nc.vector.reduce_sum(sq_ic[:], sqr[:], axis=mybir.AxisListType.X)
                    nc.vector.tensor_scalar_mul(sq_ic[:], sq_ic[:], c_temp)

                    # ---------- gumbel ----------
                    qt = pb.tile([128, N], f32, tag="qt")
                    nc.sync.dma_start(qt[:], q_d[b, t * 128:(t + 1) * 128])
                    tt = pb.tile([128, N], f32, tag="ttlq")
                    nc.scalar.activation(tt[:], qt[:], ACT.Ln, bias=c1e8[:])
                    g = pb.tile([128, N], f32, tag="g")
                    nc.scalar.activation(g[:], tt[:], ACT.Ln, scale=-1.0)

                    # ---------- matmul + combine, two 2048 halves ----------
                    lq = pb.tile([128, N], f32, tag="ttlq")
                    for h in range(2):
                        ps = pp.tile([128, 2048], f32, tag="mm")
                        h0 = h * 2048
                        for k in range(4):
                            lsl = slice(k * 512, (k + 1) * 512)
                            gsl = slice(h0 + k * 512, h0 + (k + 1) * 512)
                            has_diag = (h0 + k * 512) <= gb < (h0 + (k + 1) * 512)
                            nc.tensor.matmul(ps[:, lsl], xcT2[:, gb:gb + 128],
                                             xcT[:, gsl], start=True, stop=False)
                            nc.tensor.matmul(ps[:, lsl], ones_row[:],
                                             negsq[:, gsl], start=False,
                                             stop=not has_diag)
                            if has_diag:
                                off = gb - h0
                                nc.tensor.matmul(ps[:, off:off + 128], eye[:],
                                                 eyeneg[:], start=False, stop=True)
                        nc.vector.tensor_tensor(lq[:, h0:h0 + 2048], ps[:],
                                                g[:, h0:h0 + 2048], ALU.subtract)

                    # ---------- top-16 ----------
                    cand = psm.tile([128, 256], f32, tag="cand")
                    for ch in range(32):
                        nc.vector.max(cand[:, ch * 8:(ch + 1) * 8],
                                      lq[:, ch * 128:(ch + 1) * 128])
                    m1 = psm.tile([128, 8], f32, tag="m1")
                    nc.vector.max(m1[:], cand[:])
                    cand2 = psm.tile([128, 256], f32, tag="cand2")
                    nc.vector.match_replace(cand2[:], m1[:], cand[:], -1e30)
                    m2 = psm.tile([128, 8], f32, tag="m2")
                    nc.vector.max(m2[:], cand2[:])

                    idx = psm.tile([128, K], u32, tag="idx")
                    nc.vector.max_index(idx[:, 0:8], m1[:], lq[:])
                    nc.vector.max_index(idx[:, 8:16], m2[:], lq[:])
                    nc.sync.dma_start(idx_d[b, t * 128:(t + 1) * 128], idx[:])

                    lp = psm.tile([128, K], f32, tag="lp")
                    nc.vector.tensor_scalar(lp[:, 0:8], m1[:], sq_ic[:], None,
                                            op0=ALU.subtract)
                    nc.vector.tensor_scalar(lp[:, 8:16], m2[:], sq_ic[:], None,
                                            op0=ALU.subtract)
                    nc.sync.dma_start(lp_d[b, t * 128:(t + 1) * 128], lp[:])

    nc.finalize()
    return nc


def _get_executor(c_temp: float):
    key = round(float(c_temp), 10)
    if key in _CACHE:
        return _CACHE[key]

    import jax
    import concourse.mybir as mybir
    from concourse.bass2jax import (install_neuronx_cc_hook, _bass_exec_p,
                                    partition_id_tensor)
    from jax.sharding import Mesh, PartitionSpec
    from jax.experimental.shard_map import shard_map

    nc = _build(c_temp)
    install_neuronx_cc_hook()

    partition_name = (nc.partition_id_tensor.name
                      if nc.partition_id_tensor else None)
    in_names, out_names, out_avals, zero_shapes = [], [], [], []
    for alloc in nc.m.functions[0].allocations:
        if not isinstance(alloc, mybir.MemoryLocationSet):
            continue
        name = alloc.memorylocations[0].name
        if alloc.kind == "ExternalInput":
            if name != partition_name:
                in_names.append(name)
        elif alloc.kind == "ExternalOutput":
            out_names.append(name)
            shape = tuple(alloc.tensor_shape)
            dtype = mybir.dt.np(alloc.dtype)
            out_avals.append(jax.core.ShapedArray(shape, dtype))
            zero_shapes.append((shape, dtype))
    n_params = len(in_names)
    all_in_names = in_names + out_names
    if partition_name is not None:
        all_in_names = all_in_names + [partition_name]
    donate = tuple(range(n_params, n_params + len(out_names)))

    def _body(*args):
        operands = list(args)
        if partition_name is not None:
            operands.append(partition_id_tensor())
        return tuple(_bass_exec_p.bind(
            *operands,
            out_avals=tuple(out_avals),
            in_names=tuple(all_in_names),
            out_names=tuple(out_names),
            lowering_input_output_aliases=(),
            sim_require_finite=True,
            sim_require_nnan=True,
            nc=nc,
        ))

    devices = jax.devices()[:NCORES]
    mesh = Mesh(np.asarray(devices), ("core",))
    in_specs = (PartitionSpec("core"),) * (n_params + len(out_names))
    out_specs = (PartitionSpec("core"),) * len(out_names)
    sharded = jax.jit(
        shard_map(_body, mesh=mesh, in_specs=in_specs, out_specs=out_specs,
                  check_rep=False),
        donate_argnums=donate, keep_unused=True)

    exe = {
        "sharded": sharded,
        "mesh": mesh,
        "in_names": in_names,
        "out_names": out_names,
        "zero_shapes": zero_shapes,
        "out_avals": out_avals,
    }
    _CACHE[key] = exe
    return exe


def _run_device(c_temp, per_core_inputs):
    exe = _get_executor(c_temp)
    concat_in = [
        np.concatenate([per_core_inputs[c][name] for c in range(NCORES)], axis=0)
        for name in exe["in_names"]
    ]
    concat_zeros = [
        np.zeros((NCORES * s[0],) + tuple(s[1:]), d)
        for (s, d) in exe["zero_shapes"]
    ]
    outs = exe["sharded"](*concat_in, *concat_zeros)
    result = []
    for i, name in enumerate(exe["out_names"]):
        a = np.asarray(outs[i])
        shp = exe["out_avals"][i].shape
        result.append((name, a.reshape(NCORES, *shp)))
    return dict(result)


def kernel(x, A, temperature, q):
    x = np.asarray(x, dtype=np.float32)
    q = np.asarray(q, dtype=np.float32)
    tau = np.float32(np.clip(np.asarray(temperature, dtype=np.float32), -4.0, 4.0))
    c_temp = float(np.exp(tau, dtype=np.float32))

    xT = np.ascontiguousarray(np.transpose(x, (0, 2, 1)))          # [B,128,N]
    eye = np.eye(128, dtype=np.float32)
    eyeneg = (-1e20 * np.eye(128)).astype(np.float32)

    per_core = []
    for c in range(NCORES):
        rows = slice(c * RPC, (c + 1) * RPC)
        # rotate the column axis so this core's diagonal block lands at
        # columns [t*128, t*128+128) — keeps the compiled program identical
        # across cores.  column j of the rotated layout = global column
        # (j + c*RPC) % N.
        xT_rot = np.roll(xT, -c * RPC, axis=2)
        q_rot = np.roll(q[:, rows, :], -c * RPC, axis=2)
        per_core.append({
            "xT": np.ascontiguousarray(xT_rot),
            "qs": np.ascontiguousarray(q_rot),
            "eye": eye,
            "eyeneg": eyeneg,
        })

    outs = _run_device(c_temp, per_core)

    lp = outs["lp"]            # [8, B, RPC, K]
    idx = outs["idx"].astype(np.int64)

    logprobs = np.zeros((B, N, K), dtype=np.float32)
    indices = np.zeros((B, N, K), dtype=np.int32)
    for c in range(NCORES):
        rows = slice(c * RPC, (c + 1) * RPC)
        logprobs[:, rows, :] = lp[c]
        # un-rotate the column indices back to global columns
        indices[:, rows, :] = ((idx[c] + c * RPC) % N).astype(np.int32)

    rows_arr = np.broadcast_to(np.arange(N, dtype=np.int32)[None, :, None],
                               (B, N, K))
    edges = np.stack((indices.reshape(B, -1), rows_arr.reshape(B, -1)), axis=-2)
    offset = (np.arange(B, dtype=np.int32) * N)[:, None, None]
    edges_sparse = np.transpose(edges + offset, (1, 0, 2)).reshape(2, -1)
    return x, edges_sparse.astype(np.int32), logprobs
